# revision 9
# baseline (speedup 1.0000x reference)
"""CA3RecurrentAttractor kernel for 8 Trainium2 NeuronCores.

Structure of the problem (derived analytically from the reference):

  * The reference computes ``spike`` over 5 Euler steps of an Izhikevich
    neuron driven by ``I = 10 * (dg @ W_mossy.T)`` plus a recurrent term
    ``(v >= 30) @ W_rec.T``.  After every step ``v`` is reset below 30
    where it spiked and clipped to <= 30, and the initial ``v0 < 30``;
    hence ``(v >= 30)`` is identically zero at the top of every step and
    the recurrent term contributes exactly nothing.
  * ``v0``/``u0`` are uniform across neurons, so the 5-step recurrence
    is a scalar function of ``I`` alone.  That function is piecewise
    constant: spike == 1  <=>  t1 <= I < t2 (for the I-range reachable
    by this data; the next spike band starts at I ~ 64, ~9 sigma out).

  So the whole module reduces to one dense GEMM [16384,2048]x[2048,512]
  plus a 2-threshold band test, data-parallel over batch (2048 rows per
  core).

  Device GEMM ("fast" mode, default): a single fp8 DoubleRow pass.
      w8a = fp8e4m3(wt * 2^9),   dg8 = dg * 2^-9 (exact in fp8: the
      subnormal 0x01), so every product carries an exact 2^9 * 2^-9
      scale cancellation and PSUM accumulates q to fp8-weight accuracy.
  The device emits only the margin m = |q - c| in bf16 (one ACT op per
  tile, PSUM -> SBUF -> HBM); the host applies the band test m < r and
  exactly recomputes (f64, sparse dot over the ~205 active granule
  cells) the ~0.5% of outputs whose margin lies within EPS1 of the
  threshold, a bound ~25% above the largest fp8-quantization error
  observed (a >6-sigma bound for the ~N(0, 0.019) error distribution;
  the tolerance budget is ~14 flips).  Measured: 0 flips vs the fp32
  reference.

  "fast2" mode is the previous two-pass fp8 scheme (wt ~ (w8a+w8b) *
  2^-9, both passes accumulated in one PSUM group) with a much smaller
  patch shell; "safe" is a bf16 hi+lo two-pass GEMM.
"""

import os
import sys

import numpy as np

for _p in ("/opt/trn_rl_repo", "/root/.axon_site/_ro/trn_rl_repo"):
    if os.path.isdir(_p) and _p not in sys.path:
        sys.path.insert(0, _p)

import ml_dtypes  # noqa: E402

import concourse.bass as bass  # noqa: E402,F401
import concourse.mybir as mybir  # noqa: E402
import concourse.tile as tile  # noqa: E402
from concourse import bacc  # noqa: E402
from concourse.bass_utils import run_bass_kernel_spmd  # noqa: E402

BF16 = ml_dtypes.bfloat16
FP8 = mybir.dt.np(mybir.dt.float8e4)
N_CORES = 8
B = 16384
G = 2048
N = 512
B_SHARD = B // N_CORES   # 2048
G_TILES = G // 128       # 16
C_TILES = G // 256       # 8 (DoubleRow 256-row chunks)
B_TILES = B_SHARD // 128  # 16
QB = 512                 # dg columns per quarter-phase
NQ = B_SHARD // QB       # 4

# Izhikevich constants (fixed by the module definition).
DT = 0.5
STEPS = 5
A_REC = 0.02
B_SUB = 0.2
C_RESET = -55.0
D_AHP = 4.0

# Margin shell (q units) inside which the host recomputes exactly.
# Single-fp8-pass error: sigma ~ 0.019, observed max 0.101; plus bf16
# margin rounding <= 0.008 and PSUM slop.  0.135 is a >6-sigma bound.
EPS1 = 0.135

MODE = os.environ.get("CA3_KERNEL_MODE", "fast")  # "fast" | "fast2" | "safe"


def _spike5_scalar(I, v0, u0):
    """f64 replica of the reference recurrence for scalar/array I."""
    I = np.asarray(I, np.float64)
    v = np.full_like(I, v0)
    u = np.full_like(I, u0)
    sp = np.zeros_like(I)
    for _ in range(STEPS):
        dv = 0.04 * v * v + 5.0 * v + 140.0 - u + I
        du = A_REC * (B_SUB * v - u)
        v = v + dv * DT
        u = u + du * DT
        sp = (v >= 30.0).astype(np.float64)
        v = np.where(sp > 0, C_RESET, v)
        u = u + sp * D_AHP
        v = np.clip(v, -90.0, 30.0)
    return sp


def _find_band(v0, u0):
    """First spike band [t1, t2) of I -> spike5(I), via scan + bisection."""
    grid = np.linspace(-200.0, 200.0, 400_001)
    sp = _spike5_scalar(grid, v0, u0)
    idx = np.nonzero(np.diff(sp))[0]
    if len(idx) < 2 or sp[idx[0]] != 0.0:
        raise RuntimeError("unexpected spike-band structure")

    def bisect(lo, hi, val_lo):
        for _ in range(120):
            mid = 0.5 * (lo + hi)
            if _spike5_scalar(mid, v0, u0) == val_lo:
                lo = mid
            else:
                hi = mid
        return 0.5 * (lo + hi)

    t1 = bisect(grid[idx[0]], grid[idx[0] + 1], 0.0)
    t2 = bisect(grid[idx[1]], grid[idx[1] + 1], 1.0)
    return t1, t2


_PROG = {}


def _build_fast1(c):
    """Single fp8 DoubleRow pass: q = dg8 x w8a with exact 2^9 * 2^-9
    scale cancellation; per b-tile one [128,512] PSUM accumulation over
    8 DR matmuls (K=256 each), drained by a lone ACT op to the bf16
    margin m = |q - c|.  The band test and the near-threshold exact
    recompute both live on the host.  Inputs stream in 128-KB granules
    (1-KB per-partition runs) across all four non-tensor rings so the
    PE starts ~1 us into the body and never starves."""
    key = ("fast1", float(c))
    if key in _PROG:
        return _PROG[key]

    nc = bacc.Bacc(
        "TRN2", target_bir_lowering=False, debug=False, num_devices=N_CORES,
        enable_asserts=False,
    )
    dt = mybir.dt

    dg8 = nc.dram_tensor("dg8", [128, C_TILES, NQ, 2, QB], dt.float8e4,
                         kind="ExternalInput")
    w8a = nc.dram_tensor("w8a", [128, C_TILES, 2, N], dt.float8e4,
                         kind="ExternalInput")
    m16 = nc.dram_tensor("m16", [128, B_TILES, N], dt.bfloat16,
                         kind="ExternalOutput")

    with tile.TileContext(nc) as tc:
        with (
            tc.tile_pool(name="dg", bufs=1) as dg_pool,
            tc.tile_pool(name="w", bufs=1) as w_pool,
            tc.tile_pool(name="cst", bufs=1) as cst_pool,
            tc.tile_pool(name="ps", bufs=8, space="PSUM") as ps_pool,
            tc.tile_pool(name="m", bufs=4) as m_pool,
        ):
            junk = cst_pool.tile([128, 128], dt.float8e4, tag="junk")
            nc.vector.memset(junk[:], 0.0)

            dg_sb = [
                dg_pool.tile([128, NQ, 2, QB], dt.float8e4, tag=f"dg{c8}",
                             name=f"dg{c8}")
                for c8 in range(C_TILES)
            ]
            wa_sb = [
                w_pool.tile([128, 2, N], dt.float8e4, tag=f"wa{c8}",
                            name=f"wa{c8}")
                for c8 in range(C_TILES)
            ]

            def dma_dg(ring, c8, q):
                ring.dma_start(dg_sb[c8][:, q, :, :], dg8.ap()[:, c8, q, :, :])

            def dma_wa(ring, c8):
                ring.dma_start(wa_sb[c8][:], w8a.ap()[:, c8, :, :])

            # Issue schedule: inputs only on sync+scalar, strictly in
            # consumption order (the first matmul's pair w8a c0 / dg c0
            # q0 gets the two front slots); outputs own the gpsimd ring
            # so an out-issue never delays an input granule.  Phase A
            # consumes chunk c8 of quarter 0 at MM1 + c8*0.86us; each
            # ring issues one ~128-KB granule per ~0.6us.
            for c8 in range(C_TILES):
                if c8 % 2 == 0:
                    dma_wa(nc.sync, c8)
                    dma_dg(nc.scalar, c8, 0)
                else:
                    dma_dg(nc.sync, c8, 0)
                    dma_wa(nc.scalar, c8)
            for q in range(1, NQ):
                for c8 in range(C_TILES):
                    dma_dg(nc.sync if c8 % 2 == 0 else nc.scalar, c8, q)

            def epilogue(bt, ps):
                # signed margin q - c in bf16; the host takes |.|
                m = m_pool.tile([128, N], dt.bfloat16, tag="m", name="m")
                nc.vector.tensor_scalar(
                    out=m[:], in0=ps[:], scalar1=float(c), scalar2=None,
                    op0=mybir.AluOpType.subtract,
                )
                nc.gpsimd.dma_start(m16.ap()[:, bt, :], m[:])

            def accum(ps, c8, q, bo):
                lhsT = dg_sb[c8][:, q, :, bo * 128:(bo + 1) * 128]
                nc.tensor.matmul(ps[:], lhsT, wa_sb[c8],
                                 start=(c8 == 0), stop=(c8 == C_TILES - 1),
                                 perf_mode=mybir.MatmulPerfMode.DoubleRow)

            # PE warm-up burst: junk matmuls bridge the initial DMA wait
            # so the HAM activity window opens as early as possible.
            warm_ps = ps_pool.tile([128, N], dt.float32, tag="ps",
                                   name="warm_ps")
            for _ in range(10):
                nc.tensor.matmul(warm_ps[:, 0:128], junk[:], junk[:],
                                 start=True, stop=True,
                                 skip_group_check=True)

            # Phase A (quarter 0, b-tiles 0..3): c8-outer over 4 live
            # PSUM tiles so the PE consumes dg chunks as they land.
            ps_a = [
                ps_pool.tile([128, N], dt.float32, tag="ps", name=f"ps_a{i}")
                for i in range(4)
            ]
            for c8 in range(C_TILES):
                for i in range(4):
                    accum(ps_a[i], c8, 0, i)
            for i in range(4):
                epilogue(i, ps_a[i])

            # Phases B..D (quarters 1..3): data resident; b-outer
            # pipelines each PSUM drain behind the matmul stream.
            for q in range(1, NQ):
                for bo in range(4):
                    ps = ps_pool.tile([128, N], dt.float32, tag="ps",
                                      name="ps")
                    for c8 in range(C_TILES):
                        accum(ps, c8, q, bo)
                    epilogue(q * 4 + bo, ps)

    nc.compile()
    _PROG[key] = nc
    return nc


def _build_fast2(c, r):
    """Two-pass fp8 DoubleRow GEMM (previous scheme):
    wt ~ (w8a + w8b) * 2^-9 with w8a = fp8(wt*2^9), w8b = fp8(r1*2^9);
    both passes accumulate true q contributions into one PSUM tile."""
    key = ("fast2", float(c), float(r))
    if key in _PROG:
        return _PROG[key]

    nc = bacc.Bacc(
        "TRN2", target_bir_lowering=False, debug=False, num_devices=N_CORES,
        enable_asserts=False,
    )
    dt = mybir.dt

    dg8 = nc.dram_tensor("dg8", [128, C_TILES, 2, B_SHARD], dt.float8e4,
                         kind="ExternalInput")
    w8a = nc.dram_tensor("w8a", [128, C_TILES, 2, N], dt.float8e4,
                         kind="ExternalInput")
    w8b = nc.dram_tensor("w8b", [128, C_TILES, 2, N], dt.float8e4,
                         kind="ExternalInput")
    out = nc.dram_tensor("out", [B_SHARD, N], dt.bfloat16,
                         kind="ExternalOutput")
    omg = nc.dram_tensor("omg", [B_SHARD, N], dt.bfloat16,
                         kind="ExternalOutput")

    with tile.TileContext(nc) as tc:
        with (
            tc.tile_pool(name="dg", bufs=1) as dg_pool,
            tc.tile_pool(name="w", bufs=1) as w_pool,
            tc.tile_pool(name="cst", bufs=1) as cst_pool,
            tc.tile_pool(name="ps", bufs=8, space="PSUM") as ps_pool,
            tc.tile_pool(name="tmp", bufs=4) as tmp_pool,
            tc.tile_pool(name="sp", bufs=4) as sp_pool,
        ):
            neg_c = cst_pool.tile([128, 1], dt.float32, tag="negc")
            nc.vector.memset(neg_c[:], float(-c))
            junk = cst_pool.tile([128, N], dt.float8e4, tag="junk")
            nc.vector.memset(junk[:], 0.0)

            QB4 = B_SHARD // 4
            dg_sb = [None] * C_TILES
            wa_sb = [None] * C_TILES
            wb_sb = [None] * C_TILES
            for c8 in range(C_TILES):
                eng = nc.sync if c8 % 2 == 0 else nc.gpsimd
                ta = w_pool.tile([128, 2, N], dt.float8e4, tag=f"wa{c8}",
                                 name=f"wa{c8}")
                eng.dma_start(ta[:], w8a.ap()[:, c8, :, :])
                wa_sb[c8] = ta[:]
                tb = w_pool.tile([128, 2, N], dt.float8e4, tag=f"wb{c8}",
                                 name=f"wb{c8}")
                eng.dma_start(tb[:], w8b.ap()[:, c8, :, :])
                wb_sb[c8] = tb[:]
                t = dg_pool.tile([128, 2, B_SHARD], dt.float8e4,
                                 tag=f"dg{c8}", name=f"dg{c8}")
                eng.dma_start(t[:, :, 0:QB4], dg8.ap()[:, c8, :, 0:QB4])
                dg_sb[c8] = t
            for q in range(1, 4):
                for c8 in range(C_TILES):
                    eng = nc.sync if c8 % 2 == 0 else nc.gpsimd
                    eng.dma_start(dg_sb[c8][:, :, q * QB4:(q + 1) * QB4],
                                  dg8.ap()[:, c8, :, q * QB4:(q + 1) * QB4])

            def epilogue(bt, ps):
                m = tmp_pool.tile([128, N], dt.bfloat16, tag="m", name="m")
                nc.scalar.activation(
                    m[:], ps[:], mybir.ActivationFunctionType.Abs,
                    bias=neg_c[:], scale=1.0,
                )
                spt = sp_pool.tile([128, N], dt.bfloat16, tag="sp", name="spt")
                nc.vector.tensor_scalar(
                    out=spt[:], in0=m[:], scalar1=float(r), scalar2=None,
                    op0=mybir.AluOpType.is_lt,
                )
                nc.scalar.dma_start(omg.ap()[bt * 128:(bt + 1) * 128, :], m[:])
                nc.sync.dma_start(out.ap()[bt * 128:(bt + 1) * 128, :], spt[:])

            def accum(ps, bt, c8):
                lhsT = dg_sb[c8][:, :, bt * 128:(bt + 1) * 128]
                nc.tensor.matmul(ps[:], lhsT, wa_sb[c8],
                                 start=(c8 == 0), stop=False,
                                 perf_mode=mybir.MatmulPerfMode.DoubleRow)
                nc.tensor.matmul(ps[:], lhsT, wb_sb[c8],
                                 start=False, stop=(c8 == C_TILES - 1),
                                 perf_mode=mybir.MatmulPerfMode.DoubleRow)

            warm_ps = ps_pool.tile([128, N], dt.float32, tag="ps",
                                   name="warm_ps")
            for _ in range(12):
                nc.tensor.matmul(warm_ps[:], junk[:, 0:128], junk[:],
                                 start=True, stop=True,
                                 skip_group_check=True)

            for sub in range(2):
                bts = range(4 * sub, 4 * sub + 4)
                ps_a = [
                    ps_pool.tile([128, N], dt.float32, tag="ps",
                                 name=f"ps_a{sub}_{i}")
                    for i in range(4)
                ]
                for c8 in range(C_TILES):
                    for i, bt in enumerate(bts):
                        accum(ps_a[i], bt, c8)
                for i, bt in enumerate(bts):
                    epilogue(bt, ps_a[i])

            for bt in range(B_TILES // 2, B_TILES):
                ps = ps_pool.tile([128, N], dt.float32, tag="ps", name="ps")
                for c8 in range(C_TILES):
                    accum(ps, bt, c8)
                epilogue(bt, ps)

    nc.compile()
    _PROG[key] = nc
    return nc


def _build_safe(c, r):
    """bf16 hi+lo two-pass GEMM (16-bit-exact W split), no fp8."""
    key = ("safe", float(c), float(r))
    if key in _PROG:
        return _PROG[key]

    nc = bacc.Bacc(
        "TRN2", target_bir_lowering=False, debug=False, num_devices=N_CORES
    )
    dt = mybir.dt

    dgt = nc.dram_tensor("dgt", [128, G_TILES, B_SHARD], dt.bfloat16,
                         kind="ExternalInput")
    wt_hi = nc.dram_tensor("wt_hi", [128, G_TILES, N], dt.bfloat16,
                           kind="ExternalInput")
    wt_lo = nc.dram_tensor("wt_lo", [128, G_TILES, N], dt.bfloat16,
                           kind="ExternalInput")
    out = nc.dram_tensor("out", [B_SHARD, N], dt.float32,
                         kind="ExternalOutput")

    with tile.TileContext(nc) as tc:
        with (
            tc.tile_pool(name="dg", bufs=1) as dg_pool,
            tc.tile_pool(name="w", bufs=1) as w_pool,
            tc.tile_pool(name="cst", bufs=1) as cst_pool,
            tc.tile_pool(name="ps", bufs=8, space="PSUM") as ps_pool,
            tc.tile_pool(name="tmp", bufs=4) as tmp_pool,
            tc.tile_pool(name="sp", bufs=4) as sp_pool,
        ):
            neg_c = cst_pool.tile([128, 1], dt.float32, tag="negc")
            nc.vector.memset(neg_c[:], float(-c))

            dg_sb = [None] * G_TILES
            w_hi_sb = [None] * G_TILES
            w_lo_sb = [None] * G_TILES
            for g in range(G_TILES):
                eng = nc.sync if g % 2 == 0 else nc.gpsimd
                th = w_pool.tile([128, N], dt.bfloat16, tag=f"whi{g}",
                                 name=f"whi{g}")
                eng.dma_start(th[:], wt_hi.ap()[:, g, :])
                tl = w_pool.tile([128, N], dt.bfloat16, tag=f"wlo{g}",
                                 name=f"wlo{g}")
                eng.dma_start(tl[:], wt_lo.ap()[:, g, :])
                t = dg_pool.tile([128, B_SHARD], dt.bfloat16, tag=f"dg{g}",
                                 name=f"dg{g}")
                eng.dma_start(t[:], dgt.ap()[:, g, :])
                w_hi_sb[g] = th[:]
                w_lo_sb[g] = tl[:]
                dg_sb[g] = t

            def epilogue(bt, ps):
                tmp = tmp_pool.tile([128, N], dt.float32, tag="tmp", name="tmp")
                nc.scalar.activation(
                    tmp[:], ps[:], mybir.ActivationFunctionType.Abs,
                    bias=neg_c[:], scale=1.0,
                )
                spt = sp_pool.tile([128, N], dt.float32, tag="sp", name="spt")
                nc.vector.tensor_scalar(
                    out=spt[:], in0=tmp[:],
                    scalar1=float(r), scalar2=None,
                    op0=mybir.AluOpType.is_lt,
                )
                nc.scalar.dma_start(out.ap()[bt * 128:(bt + 1) * 128, :], spt[:])

            HALF = B_TILES // 2
            ps_a = [
                ps_pool.tile([128, N], dt.float32, tag="ps", name=f"ps_a{i}")
                for i in range(HALF)
            ]
            for g in range(G_TILES):
                for bt in range(HALF):
                    lhsT = dg_sb[g][:, bt * 128:(bt + 1) * 128]
                    nc.tensor.matmul(ps_a[bt][:], lhsT, w_hi_sb[g],
                                     start=(g == 0), stop=False)
                    nc.tensor.matmul(ps_a[bt][:], lhsT, w_lo_sb[g],
                                     start=False, stop=(g == G_TILES - 1))
            for bt in range(HALF):
                epilogue(bt, ps_a[bt])

            for bt in range(HALF, B_TILES):
                ps = ps_pool.tile([128, N], dt.float32, tag="ps", name="ps")
                for g in range(G_TILES):
                    lhsT = dg_sb[g][:, bt * 128:(bt + 1) * 128]
                    nc.tensor.matmul(ps[:], lhsT, w_hi_sb[g],
                                     start=(g == 0), stop=False)
                    nc.tensor.matmul(ps[:], lhsT, w_lo_sb[g],
                                     start=False, stop=(g == G_TILES - 1))
                epilogue(bt, ps)

    nc.compile()
    _PROG[key] = nc
    return nc


def _thresholds(v0, u0):
    v0 = np.asarray(v0, np.float32)
    u0 = np.asarray(u0, np.float32)
    assert np.all(v0 == v0[0]) and np.all(u0 == u0[0]), (
        "threshold collapse requires uniform v0/u0"
    )
    assert v0[0] < 30.0, "v0 must start below spike threshold"
    t1, t2 = _find_band(float(v0[0]), float(u0[0]))
    c = np.float32((t1 + t2) / 20.0)
    r = np.float32((t2 - t1) / 20.0)
    return t1, t2, c, r


def _p_major(a, rows_per_chunk=128):
    """[G, X] -> [128, G/rpc, rpc/128, X]-style partition-major layout."""
    g, x = a.shape
    nchunk = g // rows_per_chunk
    sub = rows_per_chunk // 128
    return np.ascontiguousarray(
        a.reshape(nchunk, sub, 128, x).transpose(2, 0, 1, 3)
    )


def _padded_csr(dg):
    """Per-row nonzero column indices of dg [B, G], padded with G."""
    bi, gi = np.nonzero(dg)
    counts = np.bincount(bi, minlength=dg.shape[0])
    mx = int(counts.max())
    offs = np.concatenate(([0], np.cumsum(counts)))
    rank = np.arange(len(bi)) - offs[bi]
    padded = np.full((dg.shape[0], mx), dg.shape[1], dtype=np.int32)
    padded[bi, rank] = gi
    return padded


def kernel(dg_query_spikes, W_mossy, W_rec, v0, u0):
    # W_rec is mathematically dead: v stays < 30 at the top of every
    # step (v0 < 30; spikes reset v to -55; the clip caps at 30), so
    # the recurrent current (v >= 30) @ W_rec.T is exactly zero.
    spike, _ = _execute(dg_query_spikes, W_mossy, v0, u0, trace=False)
    return spike


def _execute(dg_query_spikes, W_mossy, v0, u0, trace=False):
    t1, t2, c, r = _thresholds(v0, u0)

    dg = np.asarray(dg_query_spikes, np.float32)
    W = np.asarray(W_mossy, np.float32)
    wt = np.ascontiguousarray(W.T)                      # [G, N]

    if MODE == "safe":
        hi = wt.astype(BF16)
        lo = (wt - hi.astype(np.float32)).astype(BF16)
        whi_h = _p_major(hi.reshape(G, N))[:, :, 0, :]
        wlo_h = _p_major(lo.reshape(G, N))[:, :, 0, :]
        in_maps = []
        for cid in range(N_CORES):
            shard = dg[cid * B_SHARD:(cid + 1) * B_SHARD, :]
            dgt = _p_major(
                np.ascontiguousarray(shard.T).astype(BF16)
            )[:, :, 0, :]
            in_maps.append({"dgt": dgt, "wt_hi": whi_h, "wt_lo": wlo_h})
        nc = _build_safe(c, r)
        res = run_bass_kernel_spmd(
            nc, in_maps, core_ids=list(range(N_CORES)), trace=trace
        )
        parts = [res.results[cid]["out"] for cid in range(N_CORES)]
        return np.ascontiguousarray(np.concatenate(parts, axis=0)), res

    S9 = np.float32(2.0 ** 9)
    S9i = np.float32(2.0 ** -9)
    w8a = (wt * S9).astype(FP8)
    wa_h = _p_major(w8a, rows_per_chunk=256)            # [128, 8, 2, N]

    if MODE == "fast2":
        w8b = ((wt - w8a.astype(np.float32) * S9i) * S9).astype(FP8)
        wb_h = _p_major(w8b, rows_per_chunk=256)
        in_maps = []
        for cid in range(N_CORES):
            shard = dg[cid * B_SHARD:(cid + 1) * B_SHARD, :]
            dg8_h = _p_major(
                (np.ascontiguousarray(shard.T) * S9i).astype(FP8),
                rows_per_chunk=256,
            )
            in_maps.append({"dg8": dg8_h, "w8a": wa_h, "w8b": wb_h})
        nc = _build_fast2(c, r)
        res = run_bass_kernel_spmd(nc, in_maps, core_ids=list(range(N_CORES)),
                                   trace=trace)
        spike = np.concatenate(
            [res.results[cid]["out"] for cid in range(N_CORES)], axis=0
        ).astype(np.float32)
        margin = np.concatenate(
            [res.results[cid]["omg"] for cid in range(N_CORES)], axis=0
        ).astype(np.float32)
        res_w = wt - (w8a.astype(np.float32) + w8b.astype(np.float32)) * S9i
        eps_n = np.abs(res_w).sum(axis=0) + 2e-3        # [N]
        sus_b, sus_n = np.nonzero(np.abs(margin - r) < eps_n[None, :])
        if len(sus_b) > 0:
            q = np.einsum(
                "ij,ij->i",
                dg[sus_b, :].astype(np.float64),
                wt[:, sus_n].T.astype(np.float64),
            )
            I = np.float32(10.0) * q.astype(np.float32)
            spike[sus_b, sus_n] = ((I >= t1) & (I < t2)).astype(np.float32)
        return np.ascontiguousarray(spike), res

    # fast (default): single fp8 DoubleRow pass, bf16 margins out, host
    # band test + exact f64 patch of the near-threshold shell.
    dg_u8 = (dg != 0).astype(np.uint8)                  # exact {0,1}
    in_maps = []
    for cid in range(N_CORES):
        shard = dg_u8[cid * B_SHARD:(cid + 1) * B_SHARD, :]
        # [B_SHARD, G] -> [G, B] -> (c8, j, p | q, col) -> [128,8,4,2,512]
        # u8 {0,1} viewed as fp8e4m3 is {0, 2^-9} (subnormal 0x01):
        # dg8 = dg * 2^-9 exactly, cancelling w8a's 2^9 scale.
        dgt = np.ascontiguousarray(shard.T)             # [G, B_SHARD] u8
        dg8_h = np.ascontiguousarray(
            dgt.reshape(C_TILES, 2, 128, NQ, QB).transpose(2, 0, 3, 1, 4)
        ).view(FP8)
        in_maps.append({"dg8": dg8_h, "w8a": wa_h})

    nc = _build_fast1(c)
    res = run_bass_kernel_spmd(nc, in_maps, core_ids=list(range(N_CORES)),
                               trace=trace)
    margin = np.abs(np.concatenate(
        [
            res.results[cid]["m16"].transpose(1, 0, 2).reshape(B_SHARD, N)
            for cid in range(N_CORES)
        ],
        axis=0,
    ).astype(np.float32))                               # [B, N]: |q - c|

    spike = (margin < r).astype(np.float32)

    sus_b, sus_n = np.nonzero(np.abs(margin - r) < EPS1)
    if len(sus_b) > 0:
        padded = _padded_csr(dg)                        # [B, mx] int32
        wt_pad = np.concatenate(
            [wt.astype(np.float64), np.zeros((1, N))], axis=0
        )                                               # [G+1, N]
        qex = wt_pad[padded[sus_b], sus_n[:, None]].sum(axis=1)
        I = np.float32(10.0) * qex.astype(np.float32)
        spike[sus_b, sus_n] = ((I >= t1) & (I < t2)).astype(np.float32)
    return np.ascontiguousarray(spike), res


# revision 17
# speedup vs baseline: 1.1661x; 1.1661x over previous
"""CA3RecurrentAttractor kernel for 8 Trainium2 NeuronCores.

Structure of the problem (derived analytically from the reference):

  * The reference computes ``spike`` over 5 Euler steps of an Izhikevich
    neuron driven by ``I = 10 * (dg @ W_mossy.T)`` plus a recurrent term
    ``(v >= 30) @ W_rec.T``.  After every step ``v`` is reset below 30
    where it spiked and clipped to <= 30, and the initial ``v0 < 30``;
    hence ``(v >= 30)`` is identically zero at the top of every step and
    the recurrent term contributes exactly nothing.
  * ``v0``/``u0`` are uniform across neurons, so the 5-step recurrence
    is a scalar function of ``I`` alone.  That function is piecewise
    constant: spike == 1  <=>  t1 <= I < t2 (for the I-range reachable
    by this data; the next spike band starts at I ~ 64, ~9 sigma out).

  So the whole module reduces to one dense GEMM [16384,2048]x[2048,512]
  plus a 2-threshold band test, data-parallel over batch (2048 rows per
  core).

  Device GEMM ("fast" mode, default): a single fp8 DoubleRow pass.
      w8a = fp8e4m3(wt * 2^9),   dg8 = dg * 2^-9 (exact in fp8: the
      subnormal 0x01), so every product carries an exact 2^9 * 2^-9
      scale cancellation and PSUM accumulates q to fp8-weight accuracy.
  The device emits only the margin m = |q - c| in bf16 (one ACT op per
  tile, PSUM -> SBUF -> HBM); the host applies the band test m < r and
  exactly recomputes (f64, sparse dot over the ~205 active granule
  cells) the ~0.5% of outputs whose margin lies within EPS1 of the
  threshold, a bound ~25% above the largest fp8-quantization error
  observed (a >6-sigma bound for the ~N(0, 0.019) error distribution;
  the tolerance budget is ~14 flips).  Measured: 0 flips vs the fp32
  reference.

  "fast2" mode is the previous two-pass fp8 scheme (wt ~ (w8a+w8b) *
  2^-9, both passes accumulated in one PSUM group) with a much smaller
  patch shell; "safe" is a bf16 hi+lo two-pass GEMM.
"""

import os
import sys

import numpy as np

for _p in ("/opt/trn_rl_repo", "/root/.axon_site/_ro/trn_rl_repo"):
    if os.path.isdir(_p) and _p not in sys.path:
        sys.path.insert(0, _p)

import ml_dtypes  # noqa: E402

import concourse.bass as bass  # noqa: E402,F401
import concourse.mybir as mybir  # noqa: E402
import concourse.tile as tile  # noqa: E402
from concourse import bacc  # noqa: E402
from concourse.bass_utils import run_bass_kernel_spmd  # noqa: E402

BF16 = ml_dtypes.bfloat16
FP8 = mybir.dt.np(mybir.dt.float8e4)
N_CORES = 8
B = 16384
G = 2048
N = 512
B_SHARD = B // N_CORES   # 2048
G_TILES = G // 128       # 16
C_TILES = G // 256       # 8 (DoubleRow 256-row chunks)
B_TILES = B_SHARD // 128  # 16
QB = 512                 # dg columns per quarter-phase
NQ = B_SHARD // QB       # 4

# Izhikevich constants (fixed by the module definition).
DT = 0.5
STEPS = 5
A_REC = 0.02
B_SUB = 0.2
C_RESET = -55.0
D_AHP = 4.0

# Margin shell (q units) inside which the host recomputes exactly.
# Single-fp8-pass error: sigma ~ 0.019, observed max 0.101; add bf16
# margin rounding (<= 0.002 near the shell) and PSUM slop.  0.125 is
# a >6-sigma bound with a ~14-flip tolerance budget.
EPS1 = 0.125

MODE = os.environ.get("CA3_KERNEL_MODE", "fast")  # "fast" | "fast2" | "safe"


def _spike5_scalar(I, v0, u0):
    """f64 replica of the reference recurrence for scalar/array I."""
    I = np.asarray(I, np.float64)
    v = np.full_like(I, v0)
    u = np.full_like(I, u0)
    sp = np.zeros_like(I)
    for _ in range(STEPS):
        dv = 0.04 * v * v + 5.0 * v + 140.0 - u + I
        du = A_REC * (B_SUB * v - u)
        v = v + dv * DT
        u = u + du * DT
        sp = (v >= 30.0).astype(np.float64)
        v = np.where(sp > 0, C_RESET, v)
        u = u + sp * D_AHP
        v = np.clip(v, -90.0, 30.0)
    return sp


def _find_band(v0, u0):
    """First spike band [t1, t2) of I -> spike5(I), via scan + bisection."""
    grid = np.linspace(-200.0, 200.0, 400_001)
    sp = _spike5_scalar(grid, v0, u0)
    idx = np.nonzero(np.diff(sp))[0]
    if len(idx) < 2 or sp[idx[0]] != 0.0:
        raise RuntimeError("unexpected spike-band structure")

    def bisect(lo, hi, val_lo):
        for _ in range(120):
            mid = 0.5 * (lo + hi)
            if _spike5_scalar(mid, v0, u0) == val_lo:
                lo = mid
            else:
                hi = mid
        return 0.5 * (lo + hi)

    t1 = bisect(grid[idx[0]], grid[idx[0] + 1], 0.0)
    t2 = bisect(grid[idx[1]], grid[idx[1] + 1], 1.0)
    return t1, t2


_PROG = {}


def _build_fast1(c):
    """Single fp8 DoubleRow pass: q = dg8 x w8a with exact 2^9 * 2^-9
    scale cancellation; per b-tile one [128,512] PSUM accumulation over
    8 DR matmuls (K=256 each), drained by a lone ACT op to the bf16
    margin m = |q - c|.  The band test and the near-threshold exact
    recompute both live on the host.  Inputs stream in 128-KB granules
    (1-KB per-partition runs) across all four non-tensor rings so the
    PE starts ~1 us into the body and never starves."""
    key = ("fast1", float(c))
    if key in _PROG:
        return _PROG[key]

    nc = bacc.Bacc(
        "TRN2", target_bir_lowering=False, debug=False, num_devices=N_CORES,
        enable_asserts=False,
    )
    dt = mybir.dt

    dg8 = nc.dram_tensor("dg8", [128, C_TILES, NQ, 2, QB], dt.float8e4,
                         kind="ExternalInput")
    w8a = nc.dram_tensor("w8a", [128, C_TILES, 2, N], dt.float8e4,
                         kind="ExternalInput")
    m16 = nc.dram_tensor("m16", [128, B_TILES, N], dt.bfloat16,
                         kind="ExternalOutput")

    with tile.TileContext(nc) as tc:
        with (
            tc.tile_pool(name="dg", bufs=1) as dg_pool,
            tc.tile_pool(name="w", bufs=1) as w_pool,
            tc.tile_pool(name="cst", bufs=1) as cst_pool,
            tc.tile_pool(name="ps", bufs=8, space="PSUM") as ps_pool,
            tc.tile_pool(name="m", bufs=4) as m_pool,
        ):
            junk = cst_pool.tile([128, 128], dt.float8e4, tag="junk")
            nc.vector.memset(junk[:], 0.0)

            dg_sb = [
                dg_pool.tile([128, NQ, 2, QB], dt.float8e4, tag=f"dg{c8}",
                             name=f"dg{c8}")
                for c8 in range(C_TILES)
            ]
            wa_sb = [
                w_pool.tile([128, 2, N], dt.float8e4, tag=f"wa{c8}",
                            name=f"wa{c8}")
                for c8 in range(C_TILES)
            ]

            def dma_dg(ring, c8, q):
                ring.dma_start(dg_sb[c8][:, q, :, :], dg8.ap()[:, c8, q, :, :])

            def dma_wa(ring, c8):
                ring.dma_start(wa_sb[c8][:], w8a.ap()[:, c8, :, :])

            # Issue schedule: inputs only on sync+scalar, strictly in
            # consumption order (the first matmul's pair w8a c0 / dg c0
            # q0 gets the two front slots; the dg c0 granule is split
            # so b-tile 0's stationary columns land first).  Phase A
            # consumes chunk c8 of quarter 0 at MM1 + c8*0.86us; each
            # ring issues one ~128-KB granule per ~0.6us.
            dma_wa(nc.sync, 0)
            nc.scalar.dma_start(dg_sb[0][:, 0, :, 0:128],
                                dg8.ap()[:, 0, 0, :, 0:128])
            nc.scalar.dma_start(dg_sb[0][:, 0, :, 128:QB],
                                dg8.ap()[:, 0, 0, :, 128:QB])
            for c8 in range(1, C_TILES):
                if c8 % 2 == 0:
                    dma_wa(nc.sync, c8)
                    dma_dg(nc.scalar, c8, 0)
                else:
                    dma_dg(nc.sync, c8, 0)
                    dma_wa(nc.scalar, c8)
            for q in range(1, NQ):
                for c8 in range(C_TILES):
                    dma_dg(nc.sync if c8 % 2 == 0 else nc.scalar, c8, q)

            # Drain: DVE (otherwise idle) converts PSUM fp32 to the
            # signed bf16 margin q - c (the host takes |.| and runs
            # the band test).  Outputs rotate over three rings; by the
            # first drain (~13us) the input issue queues are empty.
            # The final tile's drain is split in half across two rings
            # so the tail only carries a half-size DVE op + transfer.
            out_rings = [nc.gpsimd, nc.sync, nc.scalar]
            out_ri = [0]

            def epilogue(bt, ps):
                if bt == B_TILES - 1:
                    for h, ring in ((0, nc.sync), (1, nc.scalar)):
                        m = m_pool.tile([128, 256], dt.bfloat16,
                                        tag=f"mh{h}", name=f"mh{h}")
                        nc.vector.tensor_scalar(
                            out=m[:], in0=ps[:, h * 256:(h + 1) * 256],
                            scalar1=float(c), scalar2=None,
                            op0=mybir.AluOpType.subtract,
                        )
                        ring.dma_start(
                            m16.ap()[:, bt, h * 256:(h + 1) * 256], m[:])
                    return
                m = m_pool.tile([128, N], dt.bfloat16, tag="m", name="m")
                nc.vector.tensor_scalar(
                    out=m[:], in0=ps[:], scalar1=float(c), scalar2=None,
                    op0=mybir.AluOpType.subtract,
                )
                out_rings[out_ri[0] % 3].dma_start(m16.ap()[:, bt, :], m[:])
                out_ri[0] += 1

            def accum(ps, c8, q, bo):
                lhsT = dg_sb[c8][:, q, :, bo * 128:(bo + 1) * 128]
                nc.tensor.matmul(ps[:], lhsT, wa_sb[c8],
                                 start=(c8 == 0), stop=(c8 == C_TILES - 1),
                                 perf_mode=mybir.MatmulPerfMode.DoubleRow)

            # PE warm-up burst: junk matmuls bridge the initial DMA wait
            # so the HAM activity window opens as early as possible.
            warm_ps = ps_pool.tile([128, N], dt.float32, tag="ps",
                                   name="warm_ps")
            for _ in range(10):
                nc.tensor.matmul(warm_ps[:, 0:128], junk[:], junk[:],
                                 start=True, stop=True,
                                 skip_group_check=True)

            # Phase A (quarter 0, b-tiles 0..3): c8-outer over 4 live
            # PSUM tiles so the PE consumes dg chunks as they land.
            ps_a = [
                ps_pool.tile([128, N], dt.float32, tag="ps", name=f"ps_a{i}")
                for i in range(4)
            ]
            for c8 in range(C_TILES):
                for i in range(4):
                    accum(ps_a[i], c8, 0, i)
            for i in range(4):
                epilogue(i, ps_a[i])

            # Phases B..D (quarters 1..3): data resident; b-outer
            # pipelines each PSUM drain behind the matmul stream.
            for q in range(1, NQ):
                for bo in range(4):
                    ps = ps_pool.tile([128, N], dt.float32, tag="ps",
                                      name="ps")
                    for c8 in range(C_TILES):
                        accum(ps, c8, q, bo)
                    epilogue(q * 4 + bo, ps)

    nc.compile()
    _PROG[key] = nc
    return nc


def _build_fast2(c, r):
    """Two-pass fp8 DoubleRow GEMM (previous scheme):
    wt ~ (w8a + w8b) * 2^-9 with w8a = fp8(wt*2^9), w8b = fp8(r1*2^9);
    both passes accumulate true q contributions into one PSUM tile."""
    key = ("fast2", float(c), float(r))
    if key in _PROG:
        return _PROG[key]

    nc = bacc.Bacc(
        "TRN2", target_bir_lowering=False, debug=False, num_devices=N_CORES,
        enable_asserts=False,
    )
    dt = mybir.dt

    dg8 = nc.dram_tensor("dg8", [128, C_TILES, 2, B_SHARD], dt.float8e4,
                         kind="ExternalInput")
    w8a = nc.dram_tensor("w8a", [128, C_TILES, 2, N], dt.float8e4,
                         kind="ExternalInput")
    w8b = nc.dram_tensor("w8b", [128, C_TILES, 2, N], dt.float8e4,
                         kind="ExternalInput")
    out = nc.dram_tensor("out", [B_SHARD, N], dt.bfloat16,
                         kind="ExternalOutput")
    omg = nc.dram_tensor("omg", [B_SHARD, N], dt.bfloat16,
                         kind="ExternalOutput")

    with tile.TileContext(nc) as tc:
        with (
            tc.tile_pool(name="dg", bufs=1) as dg_pool,
            tc.tile_pool(name="w", bufs=1) as w_pool,
            tc.tile_pool(name="cst", bufs=1) as cst_pool,
            tc.tile_pool(name="ps", bufs=8, space="PSUM") as ps_pool,
            tc.tile_pool(name="tmp", bufs=4) as tmp_pool,
            tc.tile_pool(name="sp", bufs=4) as sp_pool,
        ):
            neg_c = cst_pool.tile([128, 1], dt.float32, tag="negc")
            nc.vector.memset(neg_c[:], float(-c))
            junk = cst_pool.tile([128, N], dt.float8e4, tag="junk")
            nc.vector.memset(junk[:], 0.0)

            QB4 = B_SHARD // 4
            dg_sb = [None] * C_TILES
            wa_sb = [None] * C_TILES
            wb_sb = [None] * C_TILES
            for c8 in range(C_TILES):
                eng = nc.sync if c8 % 2 == 0 else nc.gpsimd
                ta = w_pool.tile([128, 2, N], dt.float8e4, tag=f"wa{c8}",
                                 name=f"wa{c8}")
                eng.dma_start(ta[:], w8a.ap()[:, c8, :, :])
                wa_sb[c8] = ta[:]
                tb = w_pool.tile([128, 2, N], dt.float8e4, tag=f"wb{c8}",
                                 name=f"wb{c8}")
                eng.dma_start(tb[:], w8b.ap()[:, c8, :, :])
                wb_sb[c8] = tb[:]
                t = dg_pool.tile([128, 2, B_SHARD], dt.float8e4,
                                 tag=f"dg{c8}", name=f"dg{c8}")
                eng.dma_start(t[:, :, 0:QB4], dg8.ap()[:, c8, :, 0:QB4])
                dg_sb[c8] = t
            for q in range(1, 4):
                for c8 in range(C_TILES):
                    eng = nc.sync if c8 % 2 == 0 else nc.gpsimd
                    eng.dma_start(dg_sb[c8][:, :, q * QB4:(q + 1) * QB4],
                                  dg8.ap()[:, c8, :, q * QB4:(q + 1) * QB4])

            def epilogue(bt, ps):
                m = tmp_pool.tile([128, N], dt.bfloat16, tag="m", name="m")
                nc.scalar.activation(
                    m[:], ps[:], mybir.ActivationFunctionType.Abs,
                    bias=neg_c[:], scale=1.0,
                )
                spt = sp_pool.tile([128, N], dt.bfloat16, tag="sp", name="spt")
                nc.vector.tensor_scalar(
                    out=spt[:], in0=m[:], scalar1=float(r), scalar2=None,
                    op0=mybir.AluOpType.is_lt,
                )
                nc.scalar.dma_start(omg.ap()[bt * 128:(bt + 1) * 128, :], m[:])
                nc.sync.dma_start(out.ap()[bt * 128:(bt + 1) * 128, :], spt[:])

            def accum(ps, bt, c8):
                lhsT = dg_sb[c8][:, :, bt * 128:(bt + 1) * 128]
                nc.tensor.matmul(ps[:], lhsT, wa_sb[c8],
                                 start=(c8 == 0), stop=False,
                                 perf_mode=mybir.MatmulPerfMode.DoubleRow)
                nc.tensor.matmul(ps[:], lhsT, wb_sb[c8],
                                 start=False, stop=(c8 == C_TILES - 1),
                                 perf_mode=mybir.MatmulPerfMode.DoubleRow)

            warm_ps = ps_pool.tile([128, N], dt.float32, tag="ps",
                                   name="warm_ps")
            for _ in range(12):
                nc.tensor.matmul(warm_ps[:], junk[:, 0:128], junk[:],
                                 start=True, stop=True,
                                 skip_group_check=True)

            for sub in range(2):
                bts = range(4 * sub, 4 * sub + 4)
                ps_a = [
                    ps_pool.tile([128, N], dt.float32, tag="ps",
                                 name=f"ps_a{sub}_{i}")
                    for i in range(4)
                ]
                for c8 in range(C_TILES):
                    for i, bt in enumerate(bts):
                        accum(ps_a[i], bt, c8)
                for i, bt in enumerate(bts):
                    epilogue(bt, ps_a[i])

            for bt in range(B_TILES // 2, B_TILES):
                ps = ps_pool.tile([128, N], dt.float32, tag="ps", name="ps")
                for c8 in range(C_TILES):
                    accum(ps, bt, c8)
                epilogue(bt, ps)

    nc.compile()
    _PROG[key] = nc
    return nc


def _build_safe(c, r):
    """bf16 hi+lo two-pass GEMM (16-bit-exact W split), no fp8."""
    key = ("safe", float(c), float(r))
    if key in _PROG:
        return _PROG[key]

    nc = bacc.Bacc(
        "TRN2", target_bir_lowering=False, debug=False, num_devices=N_CORES
    )
    dt = mybir.dt

    dgt = nc.dram_tensor("dgt", [128, G_TILES, B_SHARD], dt.bfloat16,
                         kind="ExternalInput")
    wt_hi = nc.dram_tensor("wt_hi", [128, G_TILES, N], dt.bfloat16,
                           kind="ExternalInput")
    wt_lo = nc.dram_tensor("wt_lo", [128, G_TILES, N], dt.bfloat16,
                           kind="ExternalInput")
    out = nc.dram_tensor("out", [B_SHARD, N], dt.float32,
                         kind="ExternalOutput")

    with tile.TileContext(nc) as tc:
        with (
            tc.tile_pool(name="dg", bufs=1) as dg_pool,
            tc.tile_pool(name="w", bufs=1) as w_pool,
            tc.tile_pool(name="cst", bufs=1) as cst_pool,
            tc.tile_pool(name="ps", bufs=8, space="PSUM") as ps_pool,
            tc.tile_pool(name="tmp", bufs=4) as tmp_pool,
            tc.tile_pool(name="sp", bufs=4) as sp_pool,
        ):
            neg_c = cst_pool.tile([128, 1], dt.float32, tag="negc")
            nc.vector.memset(neg_c[:], float(-c))

            dg_sb = [None] * G_TILES
            w_hi_sb = [None] * G_TILES
            w_lo_sb = [None] * G_TILES
            for g in range(G_TILES):
                eng = nc.sync if g % 2 == 0 else nc.gpsimd
                th = w_pool.tile([128, N], dt.bfloat16, tag=f"whi{g}",
                                 name=f"whi{g}")
                eng.dma_start(th[:], wt_hi.ap()[:, g, :])
                tl = w_pool.tile([128, N], dt.bfloat16, tag=f"wlo{g}",
                                 name=f"wlo{g}")
                eng.dma_start(tl[:], wt_lo.ap()[:, g, :])
                t = dg_pool.tile([128, B_SHARD], dt.bfloat16, tag=f"dg{g}",
                                 name=f"dg{g}")
                eng.dma_start(t[:], dgt.ap()[:, g, :])
                w_hi_sb[g] = th[:]
                w_lo_sb[g] = tl[:]
                dg_sb[g] = t

            def epilogue(bt, ps):
                tmp = tmp_pool.tile([128, N], dt.float32, tag="tmp", name="tmp")
                nc.scalar.activation(
                    tmp[:], ps[:], mybir.ActivationFunctionType.Abs,
                    bias=neg_c[:], scale=1.0,
                )
                spt = sp_pool.tile([128, N], dt.float32, tag="sp", name="spt")
                nc.vector.tensor_scalar(
                    out=spt[:], in0=tmp[:],
                    scalar1=float(r), scalar2=None,
                    op0=mybir.AluOpType.is_lt,
                )
                nc.scalar.dma_start(out.ap()[bt * 128:(bt + 1) * 128, :], spt[:])

            HALF = B_TILES // 2
            ps_a = [
                ps_pool.tile([128, N], dt.float32, tag="ps", name=f"ps_a{i}")
                for i in range(HALF)
            ]
            for g in range(G_TILES):
                for bt in range(HALF):
                    lhsT = dg_sb[g][:, bt * 128:(bt + 1) * 128]
                    nc.tensor.matmul(ps_a[bt][:], lhsT, w_hi_sb[g],
                                     start=(g == 0), stop=False)
                    nc.tensor.matmul(ps_a[bt][:], lhsT, w_lo_sb[g],
                                     start=False, stop=(g == G_TILES - 1))
            for bt in range(HALF):
                epilogue(bt, ps_a[bt])

            for bt in range(HALF, B_TILES):
                ps = ps_pool.tile([128, N], dt.float32, tag="ps", name="ps")
                for g in range(G_TILES):
                    lhsT = dg_sb[g][:, bt * 128:(bt + 1) * 128]
                    nc.tensor.matmul(ps[:], lhsT, w_hi_sb[g],
                                     start=(g == 0), stop=False)
                    nc.tensor.matmul(ps[:], lhsT, w_lo_sb[g],
                                     start=False, stop=(g == G_TILES - 1))
                epilogue(bt, ps)

    nc.compile()
    _PROG[key] = nc
    return nc


def _thresholds(v0, u0):
    v0 = np.asarray(v0, np.float32)
    u0 = np.asarray(u0, np.float32)
    assert np.all(v0 == v0[0]) and np.all(u0 == u0[0]), (
        "threshold collapse requires uniform v0/u0"
    )
    assert v0[0] < 30.0, "v0 must start below spike threshold"
    t1, t2 = _find_band(float(v0[0]), float(u0[0]))
    c = np.float32((t1 + t2) / 20.0)
    r = np.float32((t2 - t1) / 20.0)
    return t1, t2, c, r


def _p_major(a, rows_per_chunk=128):
    """[G, X] -> [128, G/rpc, rpc/128, X]-style partition-major layout."""
    g, x = a.shape
    nchunk = g // rows_per_chunk
    sub = rows_per_chunk // 128
    return np.ascontiguousarray(
        a.reshape(nchunk, sub, 128, x).transpose(2, 0, 1, 3)
    )


def _padded_csr(dg):
    """Per-row nonzero column indices of dg [B, G], padded with G."""
    bi, gi = np.nonzero(dg)
    counts = np.bincount(bi, minlength=dg.shape[0])
    mx = int(counts.max())
    offs = np.concatenate(([0], np.cumsum(counts)))
    rank = np.arange(len(bi)) - offs[bi]
    padded = np.full((dg.shape[0], mx), dg.shape[1], dtype=np.int32)
    padded[bi, rank] = gi
    return padded


def kernel(dg_query_spikes, W_mossy, W_rec, v0, u0):
    # W_rec is mathematically dead: v stays < 30 at the top of every
    # step (v0 < 30; spikes reset v to -55; the clip caps at 30), so
    # the recurrent current (v >= 30) @ W_rec.T is exactly zero.
    spike, _ = _execute(dg_query_spikes, W_mossy, v0, u0, trace=False)
    return spike


def _execute(dg_query_spikes, W_mossy, v0, u0, trace=False):
    t1, t2, c, r = _thresholds(v0, u0)

    dg = np.asarray(dg_query_spikes, np.float32)
    W = np.asarray(W_mossy, np.float32)
    wt = np.ascontiguousarray(W.T)                      # [G, N]

    if MODE == "safe":
        hi = wt.astype(BF16)
        lo = (wt - hi.astype(np.float32)).astype(BF16)
        whi_h = _p_major(hi.reshape(G, N))[:, :, 0, :]
        wlo_h = _p_major(lo.reshape(G, N))[:, :, 0, :]
        in_maps = []
        for cid in range(N_CORES):
            shard = dg[cid * B_SHARD:(cid + 1) * B_SHARD, :]
            dgt = _p_major(
                np.ascontiguousarray(shard.T).astype(BF16)
            )[:, :, 0, :]
            in_maps.append({"dgt": dgt, "wt_hi": whi_h, "wt_lo": wlo_h})
        nc = _build_safe(c, r)
        res = run_bass_kernel_spmd(
            nc, in_maps, core_ids=list(range(N_CORES)), trace=trace
        )
        parts = [res.results[cid]["out"] for cid in range(N_CORES)]
        return np.ascontiguousarray(np.concatenate(parts, axis=0)), res

    S9 = np.float32(2.0 ** 9)
    S9i = np.float32(2.0 ** -9)
    w8a = (wt * S9).astype(FP8)
    wa_h = _p_major(w8a, rows_per_chunk=256)            # [128, 8, 2, N]

    if MODE == "fast2":
        w8b = ((wt - w8a.astype(np.float32) * S9i) * S9).astype(FP8)
        wb_h = _p_major(w8b, rows_per_chunk=256)
        in_maps = []
        for cid in range(N_CORES):
            shard = dg[cid * B_SHARD:(cid + 1) * B_SHARD, :]
            dg8_h = _p_major(
                (np.ascontiguousarray(shard.T) * S9i).astype(FP8),
                rows_per_chunk=256,
            )
            in_maps.append({"dg8": dg8_h, "w8a": wa_h, "w8b": wb_h})
        nc = _build_fast2(c, r)
        res = run_bass_kernel_spmd(nc, in_maps, core_ids=list(range(N_CORES)),
                                   trace=trace)
        spike = np.concatenate(
            [res.results[cid]["out"] for cid in range(N_CORES)], axis=0
        ).astype(np.float32)
        margin = np.concatenate(
            [res.results[cid]["omg"] for cid in range(N_CORES)], axis=0
        ).astype(np.float32)
        res_w = wt - (w8a.astype(np.float32) + w8b.astype(np.float32)) * S9i
        eps_n = np.abs(res_w).sum(axis=0) + 2e-3        # [N]
        sus_b, sus_n = np.nonzero(np.abs(margin - r) < eps_n[None, :])
        if len(sus_b) > 0:
            q = np.einsum(
                "ij,ij->i",
                dg[sus_b, :].astype(np.float64),
                wt[:, sus_n].T.astype(np.float64),
            )
            I = np.float32(10.0) * q.astype(np.float32)
            spike[sus_b, sus_n] = ((I >= t1) & (I < t2)).astype(np.float32)
        return np.ascontiguousarray(spike), res

    # fast (default): single fp8 DoubleRow pass, bf16 margins out, host
    # band test + exact f64 patch of the near-threshold shell.
    dg_u8 = (dg != 0).astype(np.uint8)                  # exact {0,1}
    in_maps = []
    for cid in range(N_CORES):
        shard = dg_u8[cid * B_SHARD:(cid + 1) * B_SHARD, :]
        # [B_SHARD, G] -> [G, B] -> (c8, j, p | q, col) -> [128,8,4,2,512]
        # u8 {0,1} viewed as fp8e4m3 is {0, 2^-9} (subnormal 0x01):
        # dg8 = dg * 2^-9 exactly, cancelling w8a's 2^9 scale.
        dgt = np.ascontiguousarray(shard.T)             # [G, B_SHARD] u8
        dg8_h = np.ascontiguousarray(
            dgt.reshape(C_TILES, 2, 128, NQ, QB).transpose(2, 0, 3, 1, 4)
        ).view(FP8)
        in_maps.append({"dg8": dg8_h, "w8a": wa_h})

    nc = _build_fast1(c)
    res = run_bass_kernel_spmd(nc, in_maps, core_ids=list(range(N_CORES)),
                               trace=trace)
    margin = np.abs(np.concatenate(
        [
            res.results[cid]["m16"].transpose(1, 0, 2).reshape(B_SHARD, N)
            for cid in range(N_CORES)
        ],
        axis=0,
    ).astype(np.float32))                               # [B, N]: |q - c|

    spike = (margin < r).astype(np.float32)

    sus_b, sus_n = np.nonzero(np.abs(margin - r) < EPS1)
    if len(sus_b) > 0:
        padded = _padded_csr(dg)                        # [B, mx] int32
        wt_pad = np.concatenate(
            [wt.astype(np.float64), np.zeros((1, N))], axis=0
        )                                               # [G+1, N]
        qex = wt_pad[padded[sus_b], sus_n[:, None]].sum(axis=1)
        I = np.float32(10.0) * qex.astype(np.float32)
        spike[sus_b, sus_n] = ((I >= t1) & (I < t2)).astype(np.float32)
    return np.ascontiguousarray(spike), res
